# revision 16
# baseline (speedup 1.0000x reference)
"""Trainium2 Bass kernel for nn_MemoryUnit (scatter_memory).

reference math (per row of x):
    logits = x @ W.T                      -> output "pre" [N, M]
    a      = softmax(logits)
    s      = a - LAM
    att_s  = relu(s) * a / (|s| + EPS)
    att    = att_s / max(sum(att_s), EPS) -> output "att" [N, M]
    out    = att @ W                      -> output "out" [N, C]

Kernel math (scale-invariant reformulation, avoids computing softmax denom):
    e    = exp(logits)            (no max subtraction; logits in [-4.2, 4.2])
    Shat = sum(e)
    thr2 = Shat*(LAM-EPS); c = Shat*EPS
    d    = max(e - thr2, c)       ( = relu(e - LAM*Shat) + c, same fp32 rounding )
    v    = 1/d                    (approx reciprocal, ~2e-6 rel)
    q~   = c*v - 1                ( = -q, q = r/(r+c) )
    w    = (e * -1) * q~ = e*q    (+ row sum l1w)
    att  = (e * -invl1) * q~      (invl1 = 1/max(l1w, c))
    out  = (w_bf16^T @ W_bf16) * invl1

Sharding: data-parallel, N rows split over 8 cores, W replicated.
"""

import numpy as np
from contextlib import ExitStack

N_FULL, M, C = 32768, 2000, 256
NCORES = 8
NS = N_FULL // NCORES  # 4096 rows per core
P = 128
LAM, EPS = 0.0025, 1e-12
MCH = 16            # mem chunks for mm2 (interleaved: mem = p*MCH + j)
MPC = M // MCH      # 125
MPAD = 2048         # padded mem dim (multiple of 128 for dma transpose)

_CACHE = {}
TRACE = False
LAST_EXEC_NS = None
LAST_TRACE = None


def _patch_tile_drain():
    """Work around walrus 'Too many sync wait commands' on the kernel-tail
    drain: spread the drain's sem waits across preceding SP nops (<=2 per
    instruction) instead of stacking them all on one Drain."""
    from concourse import tile as _tile
    import bass_rust
    from concourse.vector_clock import ScopedClock

    if getattr(_tile.TileContext, "_drain_waits_chunked", False):
        return

    def patched(self, tick_clock, wait_clock):
        nc = self.nc
        nop0 = nc.sync.nop(nofuse=True)
        wait_clock.add_sem_waits(
            nop0.ins, ScopedClock({None: tick_clock.global_clock})
        )
        si = nop0.ins.sync_info
        waits = list(si.on_wait or []) if si is not None else []
        if len(waits) > 1:
            si.on_wait = waits[:1]
            rest = waits[1:]
            while rest:
                nopk = nc.sync.nop(nofuse=True)
                nopk.ins.sync_info = bass_rust.SyncInfo(
                    on_wait=rest[:1], on_update=[]
                )
                rest = rest[1:]
        nc.sync.drain()
        nc.all_engine_barrier()
        assert self.sems is not None
        popped = nc._tile_sem_poison_stack.pop()
        assert popped is self._sem_poison
        nc.clear_and_free_semaphores(list(self.sems.allocated().values()))
        nc.all_engine_barrier()

    _tile.TileContext._drain_and_barrier = patched
    _tile.TileContext._drain_waits_chunked = True


def _split_waits(nc, max_waits=1):
    """Walrus in this container rejects instructions carrying more than ~1
    sync wait ('Too many sync wait commands'). Hoist extra waits onto
    preceding same-engine NoOps (engine program order = bb order)."""
    import concourse.mybir as mybir
    import bass_rust

    k = 0
    for f in nc.m.functions:
        for bb in f.blocks:
            insts = list(bb.instructions)
            out = []
            changed = False
            for inst in insts:
                si = inst.sync_info
                waits = list(si.on_wait) if si is not None and si.on_wait else []
                if len(waits) > max_waits:
                    keep = waits[-max_waits:]
                    for w in waits[:-max_waits]:
                        nop = mybir.InstNoOp(name=f"waitsplit-{k}", ins=[], outs=[])
                        k += 1
                        nop.engine = inst.engine
                        nop.sync_info = bass_rust.SyncInfo(
                            on_wait=[w], on_update=[]
                        )
                        out.append(nop)
                    si.on_wait = keep
                    changed = True
                out.append(inst)
            if changed:
                try:
                    bb.instructions = out
                except Exception:
                    bb.instructions.clear()
                    bb.instructions.extend(out)
    return nc


def _build(ns, debug=False, split=True):
    import concourse.bass as bass
    import concourse.mybir as mybir
    from concourse import tile

    _patch_tile_drain()

    f32 = mybir.dt.float32
    f32r = mybir.dt.float32r
    bf16 = mybir.dt.bfloat16
    i32 = mybir.dt.int32
    AF = mybir.ActivationFunctionType
    OP = mybir.AluOpType
    ts = bass.ts

    nt = ns // P
    nc = bass.Bass("TRN2", target_bir_lowering=False, debug=False)

    x_d = nc.dram_tensor("x", [ns, C], f32, kind="ExternalInput").ap()
    w_d = nc.dram_tensor("wmat", [M, C], f32, kind="ExternalInput").ap()
    out_d = nc.dram_tensor("out", [ns, C], f32, kind="ExternalOutput").ap()
    att_d = nc.dram_tensor("att", [ns, M], f32, kind="ExternalOutput").ap()
    pre_d = nc.dram_tensor("pre", [ns, M], f32, kind="ExternalOutput").ap()
    if debug:
        e_d = nc.dram_tensor("e_dbg", [ns, M], f32, kind="ExternalOutput").ap()
        q_d = nc.dram_tensor("q_dbg", [ns, M], f32, kind="ExternalOutput").ap()
        s_d = nc.dram_tensor("s_dbg", [ns, 2], f32, kind="ExternalOutput").ap()
        l_d = nc.dram_tensor("l_dbg", [ns, 1], f32, kind="ExternalOutput").ap()

    with tile.TileContext(nc) as tc, ExitStack() as ctx:
        const = ctx.enter_context(tc.tile_pool(name="const", bufs=1))
        io = ctx.enter_context(tc.tile_pool(name="io", bufs=3))
        big = ctx.enter_context(tc.tile_pool(name="big", bufs=2))
        b16 = ctx.enter_context(tc.tile_pool(name="b16", bufs=2))
        sm = ctx.enter_context(tc.tile_pool(name="sm", bufs=2))
        ps_l = ctx.enter_context(tc.tile_pool(name="ps_l", bufs=2, space="PSUM"))
        ps_x = ctx.enter_context(tc.tile_pool(name="ps_x", bufs=2, space="PSUM"))
        ps_o = ctx.enter_context(tc.tile_pool(name="ps_o", bufs=2, space="PSUM"))

        # ---------------- one-time setup ----------------
        # identity matrix for PE transposes
        ident_i = const.tile([P, P], i32)
        nc.gpsimd.iota(ident_i[:], pattern=[[1, P]], base=0, channel_multiplier=-1)
        ident = const.tile([P, P], f32)
        nc.vector.tensor_scalar(
            out=ident[:], in0=ident_i[:], scalar1=0, scalar2=None, op0=OP.is_equal
        )

        # W staged naturally: chunk t = rows [t*125, (t+1)*125)
        w_stage = const.tile([MPC, MCH, C], f32)
        nc.sync.dma_start(w_stage[:], w_d.rearrange("(t p) c -> p t c", p=MPC))

        # WT[c, m] = W[m, kc*128 + c] as bf16 hi/lo pair (fp32 = hi + lo to
        # ~2^-17): mm1 runs 3 bf16 matmul terms for near-fp32 logits.
        wT_hi = const.tile([P, 2, M], bf16)
        wT_lo = const.tile([P, 2, M], bf16)
        for t in range(MCH):
            for kc in range(2):
                pswt_t = ps_x.tile([P, C], f32, tag="psx")
                pswt = pswt_t[:, :MPC]
                nc.tensor.transpose(
                    pswt, w_stage[:, t, ts(kc, P)], ident[:MPC, :MPC]
                )
                msl = slice(t * MPC, (t + 1) * MPC)
                nc.scalar.copy(out=wT_hi[:, kc, msl], in_=pswt)
                nc.vector.tensor_tensor(
                    out=wT_lo[:, kc, msl], in0=pswt, in1=wT_hi[:, kc, msl],
                    op=OP.subtract,
                )

        # W in bf16, contiguous 128-row chunks: w16p[p, j, c] = W[j*128 + p, c]
        # (dma_start_transpose 3D dst layout: dst[p, j, r] = src[r, j*128+p])
        w16p = const.tile([P, MCH, C], bf16)
        nc.vector.memset(w16p[:], 0.0)
        nc.gpsimd.dma_start(
            w16p[:, 0:15, :], w_d[0:15 * P, :].rearrange("(j p) c -> p j c", p=P)
        )
        nc.gpsimd.dma_start(w16p[0:M - 15 * P, 15, :], w_d[15 * P:M, :])

        # ---------------- per row-tile pipeline ----------------
        for i in range(nt):
            # load x tile, build xT via PE transpose
            x_t = io.tile([P, C], f32, tag="x")
            nc.sync.dma_start(x_t[:], x_d[ts(i, P), :])
            ps_xt = ps_x.tile([P, C], f32, tag="psx")
            for kc in range(2):
                nc.tensor.transpose(
                    ps_xt[:, ts(kc, P)], x_t[:, ts(kc, P)], ident[:]
                )
            xT_hi = sm.tile([P, C], bf16, tag="xTh")
            xT_lo = sm.tile([P, C], bf16, tag="xTl")
            nc.scalar.copy(out=xT_hi[:], in_=ps_xt[:])
            nc.vector.tensor_tensor(
                out=xT_lo[:], in0=ps_xt[:], in1=xT_hi[:], op=OP.subtract,
            )

            # mm1: logits[row, m] in two 1000-wide halves (2 psum banks each)
            e = big.tile([P, M], f32, tag="e")
            pre_s = big.tile([P, M], f32, tag="pre")
            s_parts = sm.tile([P, 2], f32, tag="sparts")
            for h in range(2):
                psl = ps_l.tile([P, 1000], f32, tag="psl")
                for n0, n1 in ((0, 512), (512, 1000)):
                    terms = []
                    for kc in range(2):
                        wsl = slice(h * 1000 + n0, h * 1000 + n1)
                        terms += [
                            (xT_hi[:, ts(kc, P)], wT_hi[:, kc, wsl]),
                            (xT_hi[:, ts(kc, P)], wT_lo[:, kc, wsl]),
                            (xT_lo[:, ts(kc, P)], wT_hi[:, kc, wsl]),
                        ]
                    for ti, (lhs, rhs) in enumerate(terms):
                        nc.tensor.matmul(
                            psl[:, n0:n1], lhs, rhs,
                            start=(ti == 0), stop=(ti == len(terms) - 1),
                        )
                hs = slice(h * 1000, (h + 1) * 1000)
                nc.scalar.activation(
                    out=e[:, hs], in_=psl[:], func=AF.Exp,
                    accum_out=s_parts[:, h:h + 1],
                )
                nc.scalar.copy(out=pre_s[:, hs], in_=psl[:])
            nc.sync.dma_start(pre_d[ts(i, P), :], pre_s[:])

            # row scalars: thr2 = Shat*(LAM-EPS), c = Shat*EPS, invc = 1/c
            thr2 = sm.tile([P, 1], f32, tag="thr2")
            c_ap = sm.tile([P, 1], f32, tag="c")
            invc = sm.tile([P, 1], f32, tag="invc")
            nc.vector.tensor_scalar(
                out=thr2[:], in0=s_parts[:, 0:1], scalar1=s_parts[:, 1:2],
                scalar2=LAM - EPS, op0=OP.add, op1=OP.mult,
            )
            nc.vector.tensor_scalar(
                out=c_ap[:], in0=s_parts[:, 0:1], scalar1=s_parts[:, 1:2],
                scalar2=EPS, op0=OP.add, op1=OP.mult,
            )
            nc.vector.reciprocal(out=invc[:], in_=c_ap[:])

            # d = max(e - thr2, c); q = 1 - c/d via exp(-ln(d/c))
            # (ln and exp are co-resident in the natural_log_exp table set;
            #  at d == c: ln(d*invc) ~ 0 -> q ~ 0 exactly)
            d = big.tile([P, M], f32, tag="d")
            nc.vector.tensor_scalar(
                out=d[:], in0=e[:], scalar1=thr2[:], scalar2=c_ap[:],
                op0=OP.subtract, op1=OP.max,
            )
            z = big.tile([P, M], f32, tag="z")
            nc.scalar.activation(out=z[:], in_=d[:], func=AF.Ln, scale=invc[:])
            v = big.tile([P, M], f32, tag="v")
            nc.scalar.activation(out=v[:], in_=z[:], func=AF.Exp, scale=-1.0)
            qt = big.tile([P, M], f32, tag="qt")
            nc.vector.tensor_scalar(
                out=qt[:], in0=v[:], scalar1=-1.0, scalar2=1.0,
                op0=OP.mult, op1=OP.add,
            )

            # w16 = e*q (bf16) with fp32 row-sum l1w
            w16 = b16.tile([P, MPAD], bf16, tag="w16")
            l1w = sm.tile([P, 1], f32, tag="l1w")
            nc.gpsimd.memset(w16[:, M:MPAD], 0.0)
            nc.vector.scalar_tensor_tensor(
                out=w16[:, 0:M], in0=e[:], scalar=1.0, in1=qt[:],
                op0=OP.mult, op1=OP.mult, accum_out=l1w[:],
            )

            # invl1p = 1/max(l1w, c)
            l1c = sm.tile([P, 1], f32, tag="l1c")
            nc.vector.tensor_scalar(
                out=l1c[:], in0=l1w[:], scalar1=c_ap[:], scalar2=None,
                op0=OP.max,
            )
            invl1p = sm.tile([P, 1], f32, tag="invl1p")
            nc.vector.reciprocal(out=invl1p[:], in_=l1c[:])

            # att = (e * invl1p) * q  (= e*q/l1)
            att_s = big.tile([P, M], f32, tag="att")
            nc.vector.scalar_tensor_tensor(
                out=att_s[:], in0=e[:], scalar=invl1p[:], in1=qt[:],
                op0=OP.mult, op1=OP.mult,
            )
            nc.sync.dma_start(att_d[ts(i, P), :], att_s[:])
            if debug:
                nc.sync.dma_start(e_d[ts(i, P), :], e[:])
                nc.sync.dma_start(q_d[ts(i, P), :], qt[:])
                nc.sync.dma_start(s_d[ts(i, P), :], s_parts[:])
                nc.sync.dma_start(l_d[ts(i, P), :], l1w[:])

            # transpose w16 via DMA xbar: wT16[p, j, row] = w16[row, p*16+j]
            wT16 = b16.tile([P, MCH, P], bf16, tag="wT16")
            nc.sync.dma_start_transpose(out=wT16[:], in_=w16[:])

            # mm2: out[row, c] = sum_j wT16[:, j, :]^T @ w16p[:, j, :]
            pso = ps_o.tile([P, C], f32, tag="pso")
            for j in range(MCH):
                nc.tensor.matmul(
                    pso[:],
                    wT16[:, j, :],
                    w16p[:, j, :],
                    start=(j == 0),
                    stop=(j == MCH - 1),
                )
            out_s = sm.tile([P, C], f32, tag="outs")
            nc.vector.tensor_scalar(
                out=out_s[:], in0=pso[:], scalar1=invl1p[:], scalar2=None,
                op0=OP.mult,
            )
            nc.sync.dma_start(out_d[ts(i, P), :], out_s[:])

    return _split_waits(nc) if split else nc


def _get_nc(ns):
    if ns not in _CACHE:
        _CACHE[ns] = _build(ns)
    return _CACHE[ns]


def kernel(x: np.ndarray, W: np.ndarray):
    from concourse.bass_utils import run_bass_kernel_spmd

    x = np.ascontiguousarray(np.asarray(x, dtype=np.float32))
    W = np.ascontiguousarray(np.asarray(W, dtype=np.float32))
    n = x.shape[0]
    ns = n // NCORES
    nc = _get_nc(ns)
    in_maps = [
        {"x": x[i * ns:(i + 1) * ns], "wmat": W} for i in range(NCORES)
    ]
    global LAST_EXEC_NS, LAST_TRACE
    res = run_bass_kernel_spmd(
        nc, in_maps, core_ids=list(range(NCORES)), trace=TRACE
    )
    LAST_EXEC_NS = res.exec_time_ns
    LAST_TRACE = res.instructions_and_trace
    out = np.concatenate([r["out"] for r in res.results], axis=0)
    att = np.concatenate([r["att"] for r in res.results], axis=0)
    pre = np.concatenate([r["pre"] for r in res.results], axis=0)
    return out, att, pre
